# revision 1
# baseline (speedup 1.0000x reference)
"""Trainium2 Bass kernel for nn_Calculator_61993557950977.

Math: for each beta, k_beta = floor(1/(1-(1-1/beta)) - 1)  (== floor(beta-1)
up to f32 rounding).  The reference's [B, dim] masked reductions collapse to

    c_j = #{b : k_beta_b > j}             (reverse cumulative histogram)
    d_j = sum_b [k_beta_b > j] * log(k_beta_b)

    ixt   = sum_j gamma_j * (d_j - log(j+1) * c_j)
    n_I   = sum_j gamma_j * c_j
    G     = sum_j gamma_j * log(lambda_j) * c_j
    H     = sum_j gamma_j * log1p(-lambda_j) * c_j

(the reference's log-ratio telescopes to log(k_beta) - log(j+1)).

On device, with j = 128*q + s (q in [0,32), s in [0,128)) and per-beta
(qb, rb) = divmod(k_beta, 128):

    c[q,s] = Cq[q] + Pc[q,s],   Cq[q]   = #{b : qb_b > q}
                                Pc[q,s] = #{b : qb_b == q and rb_b > s}
    d[q,s] = Dq[q] + Pd[q,s]    (same with log(k_beta) weights)

Pc/Pd/Cq/Dq come from one-hot / step masks contracted over the batch on the
tensor engine (PSUM accumulation).  Then sum_j u_j*c_j = sum(u .* Pc) +
sum_q Cq[q]*rowsum(u)[q], evaluated with elementwise ops + one tiny matmul.

Batch (8192) is sharded 1024 per core across 8 cores; each core emits a [3,9]
tile of partial sums; the host adds 8*5 scalars and applies the final
closed-form scalar formula.
"""

import os
import sys

for _p in ("/opt/trn_rl_repo",):
    if os.path.isdir(_p) and _p not in sys.path:
        sys.path.insert(0, _p)

import numpy as np

# Module constants from the reference nn.Module
IXY = 1.0
HX = 10.0
ALPHA = 2.0
C = 1.0
DIM = 4096
B = 8192

N_CORES = 8
BS = B // N_CORES          # betas per core
NT = BS // 128             # 8 batch tiles of 128 per core
NQ = 32                    # coarse bins  (DIM = NQ * GR)
GR = 128                   # fine bins per coarse bin

_CACHE = {}


def _build_nc():
    import concourse.bacc as bacc
    import concourse.bass as bass
    import concourse.tile as tile
    from concourse import mybir

    f32 = mybir.dt.float32
    i32 = mybir.dt.int32
    Alu = mybir.AluOpType
    ACT = mybir.ActivationFunctionType
    AX = mybir.AxisListType

    nc = bacc.Bacc("TRN2", target_bir_lowering=False, debug=False)

    betas_t = nc.dram_tensor("betas", [BS], f32, kind="ExternalInput")
    lambdas_t = nc.dram_tensor("lambdas", [DIM], f32, kind="ExternalInput")
    gammas_t = nc.dram_tensor("gammas", [DIM], f32, kind="ExternalInput")
    iq_t = nc.dram_tensor("iota_q", [128, NQ], f32, kind="ExternalInput")
    ir_t = nc.dram_tensor("iota_r", [128, GR], f32, kind="ExternalInput")
    lnj_t = nc.dram_tensor("lnj", [NQ, GR], f32, kind="ExternalInput")
    out_t = nc.dram_tensor("out", [3, 9], f32, kind="ExternalOutput")

    def bc_mid(ap, n):
        # [P, F] -> [P, n, F] with stride-0 middle dim
        return bass.AP(tensor=ap.tensor, offset=ap.offset,
                       ap=[ap.ap[0], [0, n], ap.ap[1]])

    def bc_last(ap, n):
        # [P, F] -> [P, F, n] with stride-0 last dim
        return bass.AP(tensor=ap.tensor, offset=ap.offset,
                       ap=[ap.ap[0], ap.ap[1], [0, n]])

    with tile.TileContext(nc) as tc:
        with tc.tile_pool(name="sb", bufs=1) as sb, \
             tc.tile_pool(name="ps", bufs=1, space="PSUM") as ps:
            beta = sb.tile([128, NT], f32)
            nc.sync.dma_start(out=beta, in_=betas_t.rearrange("(p t) -> p t", p=128))
            iq = sb.tile([128, NQ], f32)
            nc.sync.dma_start(out=iq, in_=iq_t[:, :])
            irr = sb.tile([128, GR], f32)
            nc.sync.dma_start(out=irr, in_=ir_t[:, :])
            lnj = sb.tile([NQ, GR], f32)
            nc.sync.dma_start(out=lnj, in_=lnj_t[:, :])
            gam = sb.tile([NQ, GR], f32)
            nc.sync.dma_start(out=gam, in_=gammas_t.rearrange("(p s) -> p s", p=NQ))
            lamt = sb.tile([NQ, GR], f32)
            nc.sync.dma_start(out=lamt, in_=lambdas_t.rearrange("(p s) -> p s", p=NQ))

            # ---- per-beta prep: k_beta, log(k_beta), q, r  ([128, NT]) ----
            r1 = sb.tile([128, NT], f32)
            nc.vector.reciprocal(r1, beta)                       # 1/beta
            lam_ = sb.tile([128, NT], f32)                       # 1 - 1/beta
            nc.vector.tensor_scalar(lam_, r1, -1.0, 1.0, op0=Alu.mult, op1=Alu.add)
            b2 = sb.tile([128, NT], f32)                         # 1 - lam
            nc.vector.tensor_scalar(b2, lam_, -1.0, 1.0, op0=Alu.mult, op1=Alu.add)
            c2 = sb.tile([128, NT], f32)
            nc.vector.reciprocal(c2, b2)                         # 1/(1-lam)
            kf = sb.tile([128, NT], f32)
            nc.vector.tensor_scalar(kf, c2, 1.0, None, op0=Alu.subtract)  # k_frac
            ki = sb.tile([128, NT], i32)
            nc.vector.tensor_copy(ki, kf)                        # cast (RNE)
            k0 = sb.tile([128, NT], f32)
            nc.vector.tensor_copy(k0, ki)
            fx = sb.tile([128, NT], f32)
            nc.vector.tensor_tensor(fx, k0, kf, op=Alu.is_gt)
            kb = sb.tile([128, NT], f32)
            nc.vector.tensor_tensor(kb, k0, fx, op=Alu.subtract)  # floor(k_frac)
            lk = sb.tile([128, NT], f32)
            nc.scalar.activation(out=lk, in_=kb, func=ACT.Ln)     # log(k_beta)
            q0 = sb.tile([128, NT], f32)
            nc.vector.tensor_scalar(q0, kb, 1.0 / GR, None, op0=Alu.mult)
            qi = sb.tile([128, NT], i32)
            nc.vector.tensor_copy(qi, q0)
            q1 = sb.tile([128, NT], f32)
            nc.vector.tensor_copy(q1, qi)
            fx2 = sb.tile([128, NT], f32)
            nc.vector.tensor_tensor(fx2, q1, q0, op=Alu.is_gt)
            qb = sb.tile([128, NT], f32)
            nc.vector.tensor_tensor(qb, q1, fx2, op=Alu.subtract)  # q = kb // GR
            qm = sb.tile([128, NT], f32)
            nc.vector.tensor_scalar(qm, qb, -float(GR), None, op0=Alu.mult)
            rb = sb.tile([128, NT], f32)
            nc.vector.tensor_tensor(rb, kb, qm, op=Alu.add)        # r = kb % GR

            # ---- masks ----
            Q = sb.tile([128, NT, NQ], f32)     # one-hot(qb)
            Tq = sb.tile([128, NT, NQ], f32)    # (q' < qb)
            Ql = sb.tile([128, NT, NQ], f32)    # one-hot * log(kb)
            rhsb = sb.tile([128, NT, 2 + GR], f32)  # [ones | log(kb) | (s < rb)]
            nc.vector.tensor_tensor(Q, bc_mid(iq, NT), bc_last(qb, NQ), op=Alu.is_equal)
            nc.vector.tensor_tensor(Tq, bc_mid(iq, NT), bc_last(qb, NQ), op=Alu.is_lt)
            nc.vector.tensor_tensor(Ql, Q, bc_last(lk, NQ), op=Alu.mult)
            nc.vector.memset(rhsb[:, :, 0:1], 1.0)
            nc.vector.tensor_copy(rhsb[:, :, 1:2], lk)
            nc.vector.tensor_tensor(rhsb[:, :, 2:], bc_mid(irr, NT), bc_last(rb, GR),
                                    op=Alu.is_lt)

            # ---- batch contraction on the tensor engine ----
            psA = ps.tile([NQ, 2 + GR], f32)    # [hist | hist*logkb | Pc]
            psB = ps.tile([NQ, GR], f32)        # Pd
            psC = ps.tile([NQ, 2], f32)         # [Cq | Dq]
            for t in range(NT):
                nc.tensor.matmul(psA, Q[:, t, :], rhsb[:, t, :],
                                 start=(t == 0), stop=(t == NT - 1))
            for t in range(NT):
                nc.tensor.matmul(psB, Ql[:, t, :], rhsb[:, t, 2:],
                                 start=(t == 0), stop=(t == NT - 1))
            for t in range(NT):
                nc.tensor.matmul(psC, Tq[:, t, :], rhsb[:, t, 0:2],
                                 start=(t == 0), stop=(t == NT - 1))

            # ---- weight tables [NQ, GR]: gamma*lnj, gamma, gamma*lnl, gamma*ln(1-l)
            lnl = sb.tile([NQ, GR], f32)
            nc.scalar.activation(out=lnl, in_=lamt, func=ACT.Ln)
            ln1m = sb.tile([NQ, GR], f32)
            nc.scalar.activation(out=ln1m, in_=lamt, func=ACT.Ln, bias=1.0, scale=-1.0)
            T4 = sb.tile([NQ, 4, GR], f32)
            nc.vector.tensor_tensor(T4[:, 0, :], lnj, gam, op=Alu.mult)
            nc.vector.tensor_copy(T4[:, 1, :], gam)
            nc.vector.tensor_tensor(T4[:, 2, :], lnl, gam, op=Alu.mult)
            nc.vector.tensor_tensor(T4[:, 3, :], ln1m, gam, op=Alu.mult)

            # ---- dot products ----
            fin_rhs = sb.tile([NQ, 9], f32)
            prods = sb.tile([NQ, 4, GR], f32)
            pc_ap = psA[:, 2:]
            pc_b = bass.AP(tensor=pc_ap.tensor, offset=pc_ap.offset,
                           ap=[pc_ap.ap[0], [0, 4], pc_ap.ap[1]])
            nc.vector.tensor_tensor(prods, T4, pc_b, op=Alu.mult)
            nc.vector.tensor_reduce(fin_rhs[:, 0:4], prods, axis=AX.X, op=Alu.add)
            prodd = sb.tile([NQ, GR], f32)
            nc.vector.tensor_tensor(prodd, psB, T4[:, 1, :], op=Alu.mult)
            nc.vector.tensor_reduce(fin_rhs[:, 4:5], prodd, axis=AX.X, op=Alu.add)
            nc.vector.tensor_reduce(fin_rhs[:, 5:9], T4, axis=AX.X, op=Alu.add)

            # ---- final partition contraction ----
            fin_lhsT = sb.tile([NQ, 3], f32)    # [ones | Cq | Dq]
            nc.vector.memset(fin_lhsT[:, 0:1], 1.0)
            nc.scalar.copy(fin_lhsT[:, 1:3], psC)
            fin_ps = ps.tile([3, 9], f32)
            nc.tensor.matmul(fin_ps, fin_lhsT, fin_rhs, start=True, stop=True)
            osb = sb.tile([3, 9], f32)
            nc.scalar.copy(osb, fin_ps)
            nc.sync.dma_start(out=out_t[:, :], in_=osb)

    nc.compile()
    return nc


def _consts():
    iq = np.broadcast_to(np.arange(NQ, dtype=np.float32), (128, NQ)).copy()
    ir = np.broadcast_to(np.arange(GR, dtype=np.float32), (128, GR)).copy()
    lnj = np.log(np.arange(1, DIM + 1, dtype=np.float64)).astype(np.float32)
    return iq, ir, lnj.reshape(NQ, GR)


def run_device(betas, lambdas, gammas, trace=False):
    from concourse.bass_utils import run_bass_kernel_spmd

    if "nc" not in _CACHE:
        _CACHE["nc"] = _build_nc()
    nc = _CACHE["nc"]

    betas = np.ascontiguousarray(np.asarray(betas, dtype=np.float32).reshape(B))
    lambdas = np.ascontiguousarray(np.asarray(lambdas, dtype=np.float32).reshape(DIM))
    gammas = np.ascontiguousarray(np.asarray(gammas, dtype=np.float32).reshape(DIM))
    iq, ir, lnj = _consts()

    in_maps = []
    for i in range(N_CORES):
        in_maps.append({
            "betas": np.ascontiguousarray(betas[i * BS:(i + 1) * BS]),
            "lambdas": lambdas,
            "gammas": gammas,
            "iota_q": iq,
            "iota_r": ir,
            "lnj": lnj,
        })

    last_err = None
    for _attempt in range(3):
        try:
            res = run_bass_kernel_spmd(nc, in_maps, core_ids=list(range(N_CORES)),
                                       trace=trace)
            break
        except Exception as e:  # transient device-recovery errors
            last_err = e
            res = None
    if res is None:
        raise last_err

    outs = np.stack([np.asarray(r["out"], dtype=np.float64) for r in res.results])
    # per-core partials -> 5 global sums
    E2 = outs[:, 0, 0] + outs[:, 1, 5]
    Nn = outs[:, 0, 1] + outs[:, 1, 6]
    G = outs[:, 0, 2] + outs[:, 1, 7]
    H = outs[:, 0, 3] + outs[:, 1, 8]
    E1 = outs[:, 0, 4] + outs[:, 2, 6]
    sums = (E1.sum(), E2.sum(), Nn.sum(), G.sum(), H.sum())
    return sums, res


def _finalize(E1, E2, Nn, G, H):
    ixt = E1 - E2
    n_I = Nn
    gm_term = np.exp(G / n_I)
    gm_comp = np.exp(H / n_I)
    exp_term = np.exp(2.0 * ixt / n_I)
    log_term = -n_I / 2.0 * np.log(gm_comp + exp_term * gm_term)
    ity = ixt + log_term
    rhs = 1.0 - ity / IXY
    lhs_1 = 1.0 - ixt / HX
    if lhs_1 < 0:
        lhs_1 = abs(lhs_1) * 20.0
    lhs = C * lhs_1 ** ALPHA
    return (np.asarray(np.float32(rhs)), np.asarray(np.float32(lhs)))


def kernel(betas, lambdas, gammas):
    sums, _ = run_device(betas, lambdas, gammas, trace=False)
    return _finalize(*sums)


# revision 2
# speedup vs baseline: 1.0406x; 1.0406x over previous
"""Trainium2 Bass kernel for nn_Calculator_61993557950977.

Math: for each beta, k_beta = floor(1/(1-(1-1/beta)) - 1)  (== floor(beta-1)
up to f32 rounding).  The reference's [B, dim] masked reductions collapse to

    c_j = #{b : k_beta_b > j}             (reverse cumulative histogram)
    d_j = sum_b [k_beta_b > j] * log(k_beta_b)

    ixt   = sum_j gamma_j * (d_j - log(j+1) * c_j)
    n_I   = sum_j gamma_j * c_j
    G     = sum_j gamma_j * log(lambda_j) * c_j
    H     = sum_j gamma_j * log1p(-lambda_j) * c_j

(the reference's log-ratio telescopes to log(k_beta) - log(j+1)).

On device, with j = 128*q + s (q in [0,32), s in [0,128)) and per-beta
(qb, rb) = divmod(k_beta, 128):

    c[q,s] = Cq[q] + Pc[q,s],   Cq[q]   = #{b : qb_b > q}
                                Pc[q,s] = #{b : qb_b == q and rb_b > s}
    d[q,s] = Dq[q] + Pd[q,s]    (same with log(k_beta) weights)

One [128,96] stationary per 128-beta tile ([onehot(q) | onehot(q)*log(kb) |
step(q)]) against a [128,130] moving tensor ([1 | log(kb) | step(r)]) gives
all of Pc/Pd/Cq/Dq in a single PSUM [96,130] accumulation over 8 tiles.
Then sum_j u_j*c_j = sum(u .* Pc) + sum_q Cq[q]*rowsum(u)[q] via elementwise
ops + one tiny matmul.

Batch (8192) is sharded 1024 per core across 8 cores; each core emits a [3,9]
tile of partial sums; the host adds 8*5 scalars and applies the final
closed-form scalar formula.
"""

import os
import sys

for _p in ("/opt/trn_rl_repo",):
    if os.path.isdir(_p) and _p not in sys.path:
        sys.path.insert(0, _p)

import numpy as np

# Module constants from the reference nn.Module
IXY = 1.0
HX = 10.0
ALPHA = 2.0
C = 1.0
DIM = 4096
B = 8192

N_CORES = 8
BS = B // N_CORES          # betas per core
NT = BS // 128             # 8 batch tiles of 128 per core
NQ = 32                    # coarse bins  (DIM = NQ * GR)
GR = 128                   # fine bins per coarse bin

_CACHE = {}


def _build_nc():
    import concourse.bacc as bacc
    import concourse.bass as bass
    import concourse.tile as tile
    from concourse import mybir

    f32 = mybir.dt.float32
    i32 = mybir.dt.int32
    Alu = mybir.AluOpType
    ACT = mybir.ActivationFunctionType
    AX = mybir.AxisListType

    nc = bacc.Bacc("TRN2", target_bir_lowering=False, debug=False)

    betas_t = nc.dram_tensor("betas", [BS], f32, kind="ExternalInput")
    lambdas_t = nc.dram_tensor("lambdas", [DIM], f32, kind="ExternalInput")
    gammas_t = nc.dram_tensor("gammas", [DIM], f32, kind="ExternalInput")
    out_t = nc.dram_tensor("out", [3, 9], f32, kind="ExternalOutput")

    def bc_mid(ap, n):
        # [P, F] -> [P, n, F] with stride-0 middle dim
        return bass.AP(tensor=ap.tensor, offset=ap.offset,
                       ap=[ap.ap[0], [0, n], ap.ap[1]])

    def bc_last(ap, n):
        # [P, F] -> [P, F, n] with stride-0 last dim
        return bass.AP(tensor=ap.tensor, offset=ap.offset,
                       ap=[ap.ap[0], ap.ap[1], [0, n]])

    with tile.TileContext(nc) as tc:
        with tc.tile_pool(name="sb", bufs=1) as sb, \
             tc.tile_pool(name="ps", bufs=1, space="PSUM") as ps:
            # ---- inputs ----
            beta = sb.tile([128, NT], f32)
            nc.sync.dma_start(out=beta, in_=betas_t.rearrange("(p t) -> p t", p=128))
            gam = sb.tile([NQ, GR], f32)
            nc.sync.dma_start(out=gam, in_=gammas_t.rearrange("(p s) -> p s", p=NQ))
            lamt = sb.tile([NQ, GR], f32)
            nc.sync.dma_start(out=lamt, in_=lambdas_t.rearrange("(p s) -> p s", p=NQ))

            # ---- constants generated on gpsimd ----
            iq_i = sb.tile([128, NQ], i32)
            nc.gpsimd.iota(iq_i, pattern=[[1, NQ]], base=0, channel_multiplier=0)
            ir_i = sb.tile([128, GR], i32)
            nc.gpsimd.iota(ir_i, pattern=[[1, GR]], base=0, channel_multiplier=0)
            ji = sb.tile([NQ, GR], i32)
            nc.gpsimd.iota(ji, pattern=[[1, GR]], base=1, channel_multiplier=GR)
            jf = sb.tile([NQ, GR], f32)
            nc.gpsimd.tensor_copy(jf, ji)           # j+1 as f32

            # ---- per-beta prep: k_beta, log(k_beta), q, r  ([128, NT]) ----
            r1 = sb.tile([128, NT], f32)
            nc.vector.reciprocal(r1, beta)                       # 1/beta
            lam_ = sb.tile([128, NT], f32)                       # 1 - 1/beta
            nc.vector.tensor_scalar(lam_, r1, -1.0, 1.0, op0=Alu.mult, op1=Alu.add)
            b2 = sb.tile([128, NT], f32)                         # 1 - lam
            nc.vector.tensor_scalar(b2, lam_, -1.0, 1.0, op0=Alu.mult, op1=Alu.add)
            c2 = sb.tile([128, NT], f32)
            nc.vector.reciprocal(c2, b2)                         # 1/(1-lam)
            kf = sb.tile([128, NT], f32)
            nc.vector.tensor_scalar(kf, c2, 1.0, None, op0=Alu.subtract)  # k_frac
            ki = sb.tile([128, NT], i32)
            nc.vector.tensor_copy(ki, kf)                        # cast (RNE)
            k0 = sb.tile([128, NT], f32)
            nc.vector.tensor_copy(k0, ki)
            fx = sb.tile([128, NT], f32)
            nc.vector.tensor_tensor(fx, k0, kf, op=Alu.is_gt)
            kb = sb.tile([128, NT], f32)
            nc.vector.tensor_tensor(kb, k0, fx, op=Alu.subtract)  # floor(k_frac)
            lk = sb.tile([128, NT], f32)
            nc.scalar.activation(out=lk, in_=kb, func=ACT.Ln)     # log(k_beta)
            kbi = sb.tile([128, NT], i32)
            nc.vector.tensor_copy(kbi, kb)                        # exact int
            qbi = sb.tile([128, NT], i32)
            nc.vector.tensor_scalar(qbi, kbi, 7, None, op0=Alu.arith_shift_right)
            rbi = sb.tile([128, NT], i32)
            nc.vector.tensor_scalar(rbi, kbi, 127, None, op0=Alu.bitwise_and)

            # ---- masks ----
            # M[:, t, :] = [onehot(qb) | onehot(qb)*log(kb) | (q' < qb)]
            M = sb.tile([128, NT, 3 * NQ], f32)
            rhsb = sb.tile([128, NT, 2 + GR], f32)  # [1 | log(kb) | (s < rb)]
            nc.vector.tensor_tensor(M[:, :, 0:NQ], bc_mid(iq_i, NT),
                                    bc_last(qbi, NQ), op=Alu.is_equal)
            nc.vector.tensor_tensor(M[:, :, 2 * NQ:3 * NQ], bc_mid(iq_i, NT),
                                    bc_last(qbi, NQ), op=Alu.is_lt)
            nc.vector.tensor_tensor(M[:, :, NQ:2 * NQ], M[:, :, 0:NQ],
                                    bc_last(lk, NQ), op=Alu.mult)
            nc.gpsimd.memset(rhsb[:, :, 0:1], 1.0)
            nc.scalar.copy(rhsb[:, :, 1:2], lk)
            nc.vector.tensor_tensor(rhsb[:, :, 2:], bc_mid(ir_i, NT),
                                    bc_last(rbi, GR), op=Alu.is_lt)

            # ---- batch contraction on the tensor engine ----
            # psum rows: [0:32]=Pc-block, [32:64]=Pd-block, [64:96]=[Cq|Dq|...]
            psum = ps.tile([3 * NQ, 2 + GR], f32)
            for t in range(NT):
                nc.tensor.matmul(psum, M[:, t, :], rhsb[:, t, :],
                                 start=(t == 0), stop=(t == NT - 1))

            # ---- weight tables [NQ, GR]: gamma*lnj, gamma, gamma*lnl, gamma*ln(1-l)
            lnl = sb.tile([NQ, GR], f32)
            nc.scalar.activation(out=lnl, in_=lamt, func=ACT.Ln)
            ln1m = sb.tile([NQ, GR], f32)
            nc.scalar.activation(out=ln1m, in_=lamt, func=ACT.Ln, bias=1.0, scale=-1.0)
            lnjl = sb.tile([NQ, GR], f32)
            nc.scalar.activation(out=lnjl, in_=jf, func=ACT.Ln)   # log(j+1)
            T4 = sb.tile([NQ, 4, GR], f32)
            nc.vector.tensor_tensor(T4[:, 0, :], lnjl, gam, op=Alu.mult)
            nc.vector.tensor_copy(T4[:, 1, :], gam)
            nc.vector.tensor_tensor(T4[:, 2, :], lnl, gam, op=Alu.mult)
            nc.vector.tensor_tensor(T4[:, 3, :], ln1m, gam, op=Alu.mult)

            # ---- dot products ----
            fin_rhs = sb.tile([NQ, 9], f32)
            prods = sb.tile([NQ, 4, GR], f32)
            pc_ap = psum[0:NQ, 2:]
            pc_b = bass.AP(tensor=pc_ap.tensor, offset=pc_ap.offset,
                           ap=[pc_ap.ap[0], [0, 4], pc_ap.ap[1]])
            nc.vector.tensor_tensor(prods, T4, pc_b, op=Alu.mult)
            nc.vector.tensor_reduce(fin_rhs[:, 0:4], prods, axis=AX.X, op=Alu.add)
            pd_sb = sb.tile([NQ, GR], f32)
            nc.scalar.copy(pd_sb, psum[NQ:2 * NQ, 2:])
            prodd = sb.tile([NQ, GR], f32)
            nc.vector.tensor_tensor(prodd, pd_sb, T4[:, 1, :], op=Alu.mult)
            nc.vector.tensor_reduce(fin_rhs[:, 4:5], prodd, axis=AX.X, op=Alu.add)
            nc.vector.tensor_reduce(fin_rhs[:, 5:9], T4, axis=AX.X, op=Alu.add)

            # ---- final partition contraction ----
            fin_lhsT = sb.tile([NQ, 3], f32)    # [ones | Cq | Dq]
            nc.gpsimd.memset(fin_lhsT[:, 0:1], 1.0)
            nc.scalar.copy(fin_lhsT[:, 1:3], psum[2 * NQ:3 * NQ, 0:2])
            fin_ps = ps.tile([3, 9], f32)
            nc.tensor.matmul(fin_ps, fin_lhsT, fin_rhs, start=True, stop=True)
            osb = sb.tile([3, 9], f32)
            nc.scalar.copy(osb, fin_ps)
            nc.sync.dma_start(out=out_t[:, :], in_=osb)

    nc.compile()
    return nc


def run_device(betas, lambdas, gammas, trace=False):
    from concourse.bass_utils import run_bass_kernel_spmd

    if "nc" not in _CACHE:
        _CACHE["nc"] = _build_nc()
    nc = _CACHE["nc"]

    betas = np.ascontiguousarray(np.asarray(betas, dtype=np.float32).reshape(B))
    lambdas = np.ascontiguousarray(np.asarray(lambdas, dtype=np.float32).reshape(DIM))
    gammas = np.ascontiguousarray(np.asarray(gammas, dtype=np.float32).reshape(DIM))

    in_maps = []
    for i in range(N_CORES):
        in_maps.append({
            "betas": np.ascontiguousarray(betas[i * BS:(i + 1) * BS]),
            "lambdas": lambdas,
            "gammas": gammas,
        })

    last_err = None
    res = None
    for _attempt in range(3):
        try:
            res = run_bass_kernel_spmd(nc, in_maps, core_ids=list(range(N_CORES)),
                                       trace=trace)
            break
        except Exception as e:  # transient device-recovery errors
            last_err = e
            res = None
    if res is None:
        raise last_err

    outs = np.stack([np.asarray(r["out"], dtype=np.float64) for r in res.results])
    # per-core partials -> 5 global sums
    E2 = outs[:, 0, 0] + outs[:, 1, 5]
    Nn = outs[:, 0, 1] + outs[:, 1, 6]
    G = outs[:, 0, 2] + outs[:, 1, 7]
    H = outs[:, 0, 3] + outs[:, 1, 8]
    E1 = outs[:, 0, 4] + outs[:, 2, 6]
    sums = (E1.sum(), E2.sum(), Nn.sum(), G.sum(), H.sum())
    return sums, res


def _finalize(E1, E2, Nn, G, H):
    ixt = E1 - E2
    n_I = Nn
    gm_term = np.exp(G / n_I)
    gm_comp = np.exp(H / n_I)
    exp_term = np.exp(2.0 * ixt / n_I)
    log_term = -n_I / 2.0 * np.log(gm_comp + exp_term * gm_term)
    ity = ixt + log_term
    rhs = 1.0 - ity / IXY
    lhs_1 = 1.0 - ixt / HX
    if lhs_1 < 0:
        lhs_1 = abs(lhs_1) * 20.0
    lhs = C * lhs_1 ** ALPHA
    return (np.asarray(np.float32(rhs)), np.asarray(np.float32(lhs)))


def kernel(betas, lambdas, gammas):
    sums, _ = run_device(betas, lambdas, gammas, trace=False)
    return _finalize(*sums)


# revision 7
# speedup vs baseline: 1.1979x; 1.1511x over previous
"""Trainium2 Bass kernel for nn_Calculator_61993557950977.

Math: for each beta, k_beta = floor(1/(1-(1-1/beta)) - 1)  (== floor(beta-1)
up to f32 rounding).  The reference's [B, dim] masked reductions collapse to

    c_j = #{b : k_beta_b > j}             (reverse cumulative histogram)
    d_j = sum_b [k_beta_b > j] * log(k_beta_b)

    ixt   = sum_j gamma_j * (d_j - log(j+1) * c_j)
    n_I   = sum_j gamma_j * c_j
    G     = sum_j gamma_j * log(lambda_j) * c_j
    H     = sum_j gamma_j * log1p(-lambda_j) * c_j

(the reference's log-ratio telescopes to log(k_beta) - log(j+1)).

On device, with j = 128*q + s (q in [0,32), s in [0,128)) and per-beta
(qb, rb) = divmod(k_beta, 128):

    c[q,s] = Cq[q] + Pc[q,s],   Cq[q]   = #{b : qb_b > q}
                                Pc[q,s] = #{b : qb_b == q and rb_b > s}
    d[q,s] = Dq[q] + Pd[q,s]    (same with log(k_beta) weights)

One bf16 [128,128] stationary per 128-beta tile ([onehot(q) | onehot*lk_hi |
onehot*lk_lo | step(q)]) against a bf16 [128,131] moving tensor ([1 | lk_hi |
lk_lo | step(r)]) gives Pc/Pd(hi+lo)/Cq/Dq(hi+lo) in one PSUM [128,131] f32
accumulation over 8 tiles (log(k_beta) is split bf16 hi+lo so products stay
exact in f32 PSUM).  Then sum_j u_j*c_j = sum(u .* Pc) + sum_q Cq*rowsum(u).

Batch (8192) is sharded 1024 per core across 8 cores; each core emits a
[32,12] tile of partial sums; the host does the final tiny (O(32)) combine
and the closed-form scalar formula.
"""

import os
import sys

for _p in ("/opt/trn_rl_repo",):
    if os.path.isdir(_p) and _p not in sys.path:
        sys.path.insert(0, _p)

import numpy as np

# Module constants from the reference nn.Module
IXY = 1.0
HX = 10.0
ALPHA = 2.0
C = 1.0
DIM = 4096
B = 8192

N_CORES = 8
BS = B // N_CORES          # betas per core
NT = BS // 128             # 8 batch tiles of 128 per core
NQ = 32                    # coarse bins  (DIM = NQ * GR)
GR = 128                   # fine bins per coarse bin
NH = NT // 2               # half of the batch tiles (PE overlap)

_CACHE = {}


def _build_nc():
    import concourse.bacc as bacc
    import concourse.bass as bass
    import concourse.tile as tile
    from concourse import mybir

    f32 = mybir.dt.float32
    i32 = mybir.dt.int32
    bf16 = mybir.dt.bfloat16
    Alu = mybir.AluOpType
    ACT = mybir.ActivationFunctionType
    AX = mybir.AxisListType

    nc = bacc.Bacc("TRN2", target_bir_lowering=False, debug=False)

    betas_t = nc.dram_tensor("betas", [BS], f32, kind="ExternalInput")
    gl_t = nc.dram_tensor("gl", [NQ, 2 * GR], f32, kind="ExternalInput")  # [gam|lam]
    out_t = nc.dram_tensor("out", [NQ, 13], f32, kind="ExternalOutput")

    def bc_mid(ap, n):
        # [P, F] -> [P, n, F] with stride-0 middle dim
        return bass.AP(tensor=ap.tensor, offset=ap.offset,
                       ap=[ap.ap[0], [0, n], ap.ap[1]])

    def bc_last(ap, n):
        # [P, F] -> [P, F, n] with stride-0 last dim
        return bass.AP(tensor=ap.tensor, offset=ap.offset,
                       ap=[ap.ap[0], ap.ap[1], [0, n]])

    with tile.TileContext(nc) as tc:
        with tc.tile_pool(name="sb", bufs=1) as sb, \
             tc.tile_pool(name="ps", bufs=1, space="PSUM") as ps:
            # ---- inputs ----
            beta = sb.tile([128, NT], f32)
            nc.sync.dma_start(out=beta, in_=betas_t.rearrange("(p t) -> p t", p=128))
            gl = sb.tile([NQ, 2, GR], f32)
            nc.sync.dma_start(out=gl, in_=gl_t.rearrange("p (k s) -> p k s", k=2))
            gam = gl[:, 0, :]
            lamt = gl[:, 1, :]

            # ---- constants generated on gpsimd ----
            iq_i = sb.tile([128, NQ], i32)
            nc.gpsimd.iota(iq_i, pattern=[[1, NQ]], base=0, channel_multiplier=0)
            ir_i = sb.tile([128, GR], i32)
            nc.gpsimd.iota(ir_i, pattern=[[1, GR]], base=0, channel_multiplier=0)
            ji = sb.tile([NQ, GR], i32)
            nc.gpsimd.iota(ji, pattern=[[1, GR]], base=1, channel_multiplier=GR)
            jf = sb.tile([NQ, GR], f32)
            nc.gpsimd.tensor_copy(jf, ji)           # j+1 as f32

            # ---- per-beta prep ([128, NT]) ----
            # k_beta = floor(beta - 1) via RNE cast of (beta - 1.5).
            kh = sb.tile([128, NT], f32)
            nc.vector.tensor_scalar(kh, beta, 1.5, None, op0=Alu.subtract)
            kbi = sb.tile([128, NT], i32)
            nc.vector.tensor_copy(kbi, kh)                       # RNE -> floor
            kbf = sb.tile([128, NT], f32)
            nc.vector.tensor_copy(kbf, kbi)
            qbi = sb.tile([128, NT], i32)
            nc.vector.tensor_scalar(qbi, kbi, 7, None, op0=Alu.arith_shift_right)
            rbi = sb.tile([128, NT], i32)
            nc.vector.tensor_scalar(rbi, kbi, 127, None, op0=Alu.bitwise_and)
            lk = sb.tile([128, NT], f32)
            nc.scalar.activation(out=lk, in_=kbf, func=ACT.Ln)   # log(k_beta)
            lkh = sb.tile([128, NT], bf16)
            nc.vector.tensor_copy(lkh, lk)                       # hi part
            lkhf = sb.tile([128, NT], f32)
            nc.vector.tensor_copy(lkhf, lkh)
            lklf = sb.tile([128, NT], f32)
            nc.vector.tensor_tensor(lklf, lk, lkhf, op=Alu.subtract)
            lkl = sb.tile([128, NT], bf16)
            nc.vector.tensor_copy(lkl, lklf)                     # lo part

            # ---- masks (bf16), built in two t-halves so PE can start early --
            # M[:, t, :] = [onehot(qb) | onehot*lk_hi | onehot*lk_lo | (q'<qb)]
            M = sb.tile([128, NT, 4 * NQ], bf16)
            rhsb = sb.tile([128, NT, 3 + GR], bf16)  # [1 | lk_hi | lk_lo | (s<rb)]
            nc.gpsimd.memset(rhsb[:, :, 0:1], 1.0)
            for h in range(2):
                ts_ = slice(h * NH, (h + 1) * NH)
                nc.vector.tensor_tensor(M[:, ts_, 0:NQ],
                                        bc_mid(iq_i, NH),
                                        bc_last(qbi[:, ts_], NQ), op=Alu.is_equal)
                nc.vector.tensor_tensor(M[:, ts_, 3 * NQ:4 * NQ],
                                        bc_mid(iq_i, NH),
                                        bc_last(qbi[:, ts_], NQ), op=Alu.is_lt)
                nc.vector.tensor_tensor(M[:, ts_, NQ:2 * NQ], M[:, ts_, 0:NQ],
                                        bc_last(lkh[:, ts_], NQ), op=Alu.mult)
                nc.vector.tensor_tensor(M[:, ts_, 2 * NQ:3 * NQ], M[:, ts_, 0:NQ],
                                        bc_last(lkl[:, ts_], NQ), op=Alu.mult)
                nc.scalar.copy(rhsb[:, ts_, 1:2], lkh[:, ts_])
                nc.scalar.copy(rhsb[:, ts_, 2:3], lkl[:, ts_])
                nc.vector.tensor_tensor(rhsb[:, ts_, 3:],
                                        bc_mid(ir_i, NH),
                                        bc_last(rbi[:, ts_], GR), op=Alu.is_lt)

            # ---- batch contraction on the tensor engine ----
            # psum rows: [0:32]=Pc, [32:64]=Pd_hi, [64:96]=Pd_lo,
            #            [96:128]=[Cq|Dq_hi|Dq_lo|...]
            psum = ps.tile([4 * NQ, 3 + GR], f32)
            for t in range(NT):
                nc.tensor.matmul(psum, M[:, t, :], rhsb[:, t, :],
                                 start=(t == 0), stop=(t == NT - 1))

            # ---- weight tables [NQ, GR] (overlap with PE) ----
            lnl = sb.tile([NQ, GR], f32)
            nc.scalar.activation(out=lnl, in_=lamt, func=ACT.Ln)
            ln1m = sb.tile([NQ, GR], f32)
            nc.scalar.activation(out=ln1m, in_=lamt, func=ACT.Ln, bias=1.0, scale=-1.0)
            lnjl = sb.tile([NQ, GR], f32)
            nc.scalar.activation(out=lnjl, in_=jf, func=ACT.Ln)   # log(j+1)
            T4 = sb.tile([NQ, 4, GR], f32)
            nc.vector.tensor_tensor(T4[:, 0, :], lnjl, gam, op=Alu.mult)
            nc.vector.tensor_copy(T4[:, 1, :], gam)
            nc.vector.tensor_tensor(T4[:, 2, :], lnl, gam, op=Alu.mult)
            nc.vector.tensor_tensor(T4[:, 3, :], ln1m, gam, op=Alu.mult)

            outsb = sb.tile([NQ, 13], f32)
            # cols 9:13 = rowsums of [g*lnj, g, g*lnl, g*ln1m]
            nc.vector.tensor_reduce(outsb[:, 9:13], T4, axis=AX.X, op=Alu.add)

            # ---- dot products against Pc / Pd ----
            prods = sb.tile([NQ, 4, GR], f32)
            pc_ap = psum[0:NQ, 3:]
            pc_b = bass.AP(tensor=pc_ap.tensor, offset=pc_ap.offset,
                           ap=[pc_ap.ap[0], [0, 4], pc_ap.ap[1]])
            nc.vector.tensor_tensor(prods, T4, pc_b, op=Alu.mult)
            nc.vector.tensor_reduce(outsb[:, 0:4], prods, axis=AX.X, op=Alu.add)
            pd2 = sb.tile([NQ, 2, GR], f32)
            nc.scalar.copy(pd2[:, 0, :], psum[NQ:2 * NQ, 3:])
            nc.scalar.copy(pd2[:, 1, :], psum[2 * NQ:3 * NQ, 3:])
            prods2 = sb.tile([NQ, 2, GR], f32)
            nc.vector.tensor_tensor(prods2, bc_mid(T4[:, 1, :], 2), pd2, op=Alu.mult)
            nc.vector.tensor_reduce(outsb[:, 4:6], prods2, axis=AX.X, op=Alu.add)
            # cols 6:9 <- raw [Cq | Dq_hi | Dq_lo]
            nc.scalar.copy(outsb[:, 6:9], psum[3 * NQ:4 * NQ, 0:3])

            nc.sync.dma_start(out=out_t[:, :], in_=outsb)

    nc.compile()
    return nc


def run_device(betas, lambdas, gammas, trace=False):
    from concourse.bass_utils import run_bass_kernel_spmd

    if "nc" not in _CACHE:
        _CACHE["nc"] = _build_nc()
    nc = _CACHE["nc"]

    betas = np.ascontiguousarray(np.asarray(betas, dtype=np.float32).reshape(B))
    lambdas = np.asarray(lambdas, dtype=np.float32).reshape(DIM)
    gammas = np.asarray(gammas, dtype=np.float32).reshape(DIM)
    gl = np.concatenate([gammas.reshape(NQ, GR), lambdas.reshape(NQ, GR)],
                        axis=1)
    gl = np.ascontiguousarray(gl)

    in_maps = []
    for i in range(N_CORES):
        in_maps.append({
            "betas": np.ascontiguousarray(betas[i * BS:(i + 1) * BS]),
            "gl": gl,
        })

    last_err = None
    res = None
    for _attempt in range(3):
        try:
            res = run_bass_kernel_spmd(nc, in_maps, core_ids=list(range(N_CORES)),
                                       trace=trace)
            break
        except Exception as e:  # transient device-recovery errors
            last_err = e
            res = None
    if res is None:
        raise last_err

    o = np.stack([np.asarray(r["out"], dtype=np.float64) for r in res.results])
    # o[:, :, c]: 0..3 = sum(T4_k .* Pc) row partials (k = g*lnj, g, g*lnl, g*ln1m)
    # 4,5 = sum(g .* Pd_hi), sum(g .* Pd_lo) row partials
    # 6,7,8 = Cq | Dq_hi | Dq_lo ; 9..12 = rowsums of [g*lnj, g, g*lnl, g*ln1m]
    Cq = o[:, :, 6]
    Dq = o[:, :, 7] + o[:, :, 8]
    E2 = (o[:, :, 0] + Cq * o[:, :, 9]).sum()
    Nn = (o[:, :, 1] + Cq * o[:, :, 10]).sum()
    G = (o[:, :, 2] + Cq * o[:, :, 11]).sum()
    H = (o[:, :, 3] + Cq * o[:, :, 12]).sum()
    E1 = (o[:, :, 4] + o[:, :, 5] + Dq * o[:, :, 10]).sum()
    sums = (E1, E2, Nn, G, H)
    return sums, res


def _finalize(E1, E2, Nn, G, H):
    ixt = E1 - E2
    n_I = Nn
    gm_term = np.exp(G / n_I)
    gm_comp = np.exp(H / n_I)
    exp_term = np.exp(2.0 * ixt / n_I)
    log_term = -n_I / 2.0 * np.log(gm_comp + exp_term * gm_term)
    ity = ixt + log_term
    rhs = 1.0 - ity / IXY
    lhs_1 = 1.0 - ixt / HX
    if lhs_1 < 0:
        lhs_1 = abs(lhs_1) * 20.0
    lhs = C * lhs_1 ** ALPHA
    return (np.asarray(np.float32(rhs)), np.asarray(np.float32(lhs)))


def kernel(betas, lambdas, gammas):
    sums, _ = run_device(betas, lambdas, gammas, trace=False)
    return _finalize(*sums)
